# revision 87
# baseline (speedup 1.0000x reference)
"""Trainium2 Bass kernel for nn_BloqueAttn: causal RoPE attention, 16 heads,
head-sharded (tensor-parallel) across 8 NeuronCores, o_proj row-sharded with
host-side all-reduce of the partials.

v2: bf16 datapath, query-on-partition PV (65-wide moving operand), PE
perm-matmul RoPE swap, mask-by-multiply on DVE, per-partition softmax
normalization, batched DMAs with host-side pre-layout.

Self-contained: hardcodes shapes B=1, L=4096, D=1024, H=16, DH=64, 8 cores.
"""
import os

os.environ.setdefault("BASS_NEVER_TRACE", "1")

import numpy as np
import ml_dtypes

import concourse.bass as bass
import concourse.bacc as bacc
import concourse.mybir as mybir
import concourse.tile as tile
from concourse.bass_utils import run_bass_kernel_spmd

F32 = mybir.dt.float32
BF16 = mybir.dt.bfloat16
I16 = mybir.dt.int16

B, L, D = 1, 4096, 1024
H, DH = 16, 64
BASE = 10000.0
N_CORES = 8
HPC = H // N_CORES          # heads per core = 2
DH2 = HPC * DH              # packed head dim = 128
SCALE = DH ** -0.5          # 0.125

# Schraudolph-style exp in bf16 bits: bf16(e^(x*SCALE)) ~= bits of
# int16(A*x + B) with A = SCALE * 2^7 / ln2, B = 127*2^7 - 7.41 (minimax).
SCH_A = SCALE * 128.0 / np.log(2.0)
SCH_B = 16256.0 - 7.41


# ---------------------------------------------------------------- host helpers

def _rope_tables(L_, dh):
    inv_freq = 1.0 / (BASE ** (np.arange(0, dh, 2, dtype=np.float32) / dh))
    freqs = np.outer(np.arange(L_, dtype=np.float32), inv_freq)  # [L, 32]
    return np.cos(freqs).astype(np.float32), np.sin(freqs).astype(np.float32)


def _host_consts(L_):
    cos, sin = _rope_tables(L_, DH)          # [L, 32]
    cosT, sinT = cos.T.copy(), sin.T.copy()  # [32, L]
    cos_stack = np.concatenate([cosT, cosT, cosT, cosT], 0)          # [128, L]
    sin_signed = np.concatenate([-sinT, sinT, -sinT, sinT], 0)       # [128, L]

    # 0/1 causal keep-mask within a 128x128 diagonal block:
    # key j visible to query c iff j <= c.
    j = np.arange(128)[:, None]
    c = np.arange(128)[None, :]
    tril01 = (j <= c).astype(np.float32)                             # [128,128]

    ident = np.eye(128, dtype=np.float32)
    # 32-row block swap permutation: out[i] = in[sigma(i)],
    # sigma = [32..63, 0..31, 96..127, 64..95]
    sigma = np.concatenate([np.arange(32, 64), np.arange(0, 32),
                            np.arange(96, 128), np.arange(64, 96)])
    pmat = np.zeros((128, 128), np.float32)
    pmat[sigma, np.arange(128)] = 1.0        # out = pmat.T @ in
    cs = np.concatenate([cos_stack, sin_signed], 1)          # [128, 2L]
    # trib: lhsT for the PE mask-bias matmul (identity moving):
    # out[m, n] = trib[n, m] = -29952 where m > n
    trib = np.triu(np.full((128, 128), -29952.0, np.float32), 1)
    tip = np.concatenate([tril01, ident, pmat, trib], 1)     # [128, 512]
    return {
        "cs": cs.astype(ml_dtypes.bfloat16),
        "tip": tip.astype(ml_dtypes.bfloat16),
    }


def _chunk_major(wT):
    """[D, 128] -> [128, D] with 128-row chunks laid side by side."""
    ndc = wT.shape[0] // 128
    return np.ascontiguousarray(
        wT.reshape(ndc, 128, 128).transpose(1, 0, 2).reshape(128, ndc * 128))


def _core_weights(core, Wq, Wk, Wv, Wo):
    """Per-core weight slices, bf16, chunk-major; RoPE even/odd permutation
    applied to Wq/Wk rows."""
    perm = np.concatenate([np.arange(0, DH, 2), np.arange(1, DH, 2)])  # [64]
    rows_p, rows = [], []
    for hh in (HPC * core, HPC * core + 1):
        rows_p.append(DH * hh + perm)
        rows.append(DH * hh + np.arange(DH))
    rows_p = np.concatenate(rows_p)
    rows = np.concatenate(rows)
    wq = _chunk_major(Wq[rows_p, :].T).astype(ml_dtypes.bfloat16)  # [128, 1024]
    wk = _chunk_major(Wk[rows_p, :].T)
    wv = _chunk_major(Wv[rows, :].T)
    woC = np.ascontiguousarray(Wo[:, DH2 * core: DH2 * (core + 1)].T)
    wkv = np.concatenate([wk, wv], 1).astype(ml_dtypes.bfloat16)
    return wq, wkv, woC.astype(ml_dtypes.bfloat16)


def _layout_x(x, L_):
    """x [B,L,D] -> [128, 8*4096] bf16, 512-col subtile-major:
    xr[p, s*4096 + ch*512 + c] = x[s*512+c, ch*128+p]."""
    xT = np.ascontiguousarray(x.reshape(L_, D).T)        # [D, L]
    ns = L_ // 512
    xr = xT.reshape(8, 128, ns, 512).transpose(1, 2, 0, 3)
    return np.ascontiguousarray(xr.reshape(128, ns * 4096)).astype(
        ml_dtypes.bfloat16)


# ---------------------------------------------------------------- device emit

def emit(nc, tc, aps, L_):
    NSB = L_ // 512           # 512-col subtiles (8) == query blocks
    NQB = L_ // 512
    NKB = L_ // 128           # key blocks (32)
    ND = D // 128             # D chunks (8)

    xt = aps["xt"]
    partial = aps["partial"]
    ACT_EXP = mybir.ActivationFunctionType.Exp

    with tc.tile_pool(name="persist", bufs=1) as pp, \
         tc.tile_pool(name="psB", bufs=1, space="PSUM") as psB, \
         tc.tile_pool(name="psS", bufs=1, space="PSUM") as psS, \
         tc.tile_pool(name="sbC", bufs=1) as sbC, \
         tc.tile_pool(name="sbB", bufs=1) as sbB, \
         tc.tile_pool(name="sbA", bufs=1) as sbA:
        wq_sb = pp.tile([128, D], BF16)
        wkvo_sb = pp.tile([128, 3 * D], BF16)
        wk_sb = wkvo_sb[:, 0:D]
        wv_sb = wkvo_sb[:, D:2 * D]
        wo_sb = wkvo_sb[:, 2 * D:3 * D]
        wkv_view = wkvo_sb[:, 0:2 * D]
        cs_sb = pp.tile([128, 2 * L_], BF16)
        cos_sb = cs_sb[:, 0:L_]
        sin_sb = cs_sb[:, L_:2 * L_]
        tip_sb = pp.tile([128, 512], BF16)
        tril_sb = tip_sb[:, 0:128]
        idb_sb = tip_sb[:, 128:256]
        perm_sb = tip_sb[:, 256:384]
        trib_sb = tip_sb[:, 384:512]
        qT = pp.tile([128, L_], BF16)
        kT = pp.tile([128, L_], BF16)
        v_sb = pp.tile([128, NKB * 130], BF16)
        # PE pstate warmup: the cost model ramps the PE clock over 3us from
        # the first matmul; dummy matmuls during the initial DMA wait start
        # the ramp early so real work runs at mid/full speed sooner. They
        # also cover the window until xt chunk 7 lands for the s=0 q-proj.
        wup = pp.tile([128, 256], BF16)
        nc.gpsimd.memset(wup[:], 0.0)
        nc.sync.dma_start(wq_sb[:], aps["wq"][:])
        ones_cols = v_sb[:].rearrange("p (kb h c) -> p kb h c",
                                      h=2, c=65)[:, :, :, 64:65]
        nc.gpsimd.memset(ones_cols, 1.0)  # ones columns for the sum trick
        for _ in range(12):
            wps = psS.tile([128, 512], F32, tag="scr", bufs=2)
            nc.tensor.matmul(wps[:, 0:256], wup[:, 0:128], wup[:],
                             start=True, stop=True)

        def phase_a_units(s):
            """Projections + RoPE + V transpose for L-subtile s, as a
            generator: each next() emits one schedulable unit so the caller
            can weave these between attention key blocks."""
            sl = bass.ds(512 * s, 512)
            xt_t = sbA.tile([128, 4096], BF16, tag="xt", bufs=3)
            if s == 0:
                # batched input streaming: HWDGE costs ~625ns per DMA, so
                # few large DMAs beat many small ones
                nc.sync.dma_start(xt_t[:, 0:2048], xt[:, 0:2048])
                nc.sync.dma_start(xt_t[:, 2048:4096], xt[:, 2048:4096])
                nc.sync.dma_start(wkv_view, aps["wkv"][:])
                nc.sync.dma_start(tip_sb[:], aps["tip"][:])
                # (wq DMA is issued before the warmup matmuls)
                # only the first 512-col slice of cos/sin is needed for s=0;
                # the rest streams in behind xt(1) to unblock it
                cs_head_d = aps["cs"][:].rearrange(
                    "p (t c) -> p t c", t=2)[:, :, 0:512]
                cs_head_s = cs_sb[:].rearrange(
                    "p (t c) -> p t c", t=2)[:, :, 0:512]
                nc.sync.dma_start(cs_head_s, cs_head_d)
            else:
                nc.sync.dma_start(xt_t[:], xt[:, bass.ts(s, 4096)])
                if s == 1:
                    nc.sync.dma_start(wo_sb[:], aps["wo"][:])
                    cs_rest_d = aps["cs"][:].rearrange(
                        "p (t c) -> p t c", t=2)[:, :, 512:L_]
                    cs_rest_s = cs_sb[:].rearrange(
                        "p (t c) -> p t c", t=2)[:, :, 512:L_]
                    nc.sync.dma_start(cs_rest_s, cs_rest_d)
            yield
            raws = {}
            # q/k projections first so their RoPE (DVE) overlaps the
            # V projection + transposes (PE) and attention(s) starts clean.
            for name, wsb in (("q", wq_sb), ("k", wk_sb)):
                ps = psS.tile([128, 512], F32, tag="scr", bufs=2)
                for ch in range(ND):
                    nc.tensor.matmul(ps[:], wsb[:, bass.ts(ch, 128)],
                                     xt_t[:, bass.ts(ch, 512)],
                                     start=ch == 0, stop=ch == ND - 1)
                    if ch == 3:
                        yield
                raw = sbA.tile([128, 512], BF16, tag=f"raw{name}", bufs=4)
                nc.scalar.copy(raw[:], ps[:])
                raws[name] = raw
                yield
            # RoPE: rot = raw*cos + perm(raw)*sin_signed. The final add is
            # SBUF-only so it rides Pool, freeing DVE for exp conversions.
            for name, dst in (("q", qT), ("k", kT)):
                raw = raws[name]
                aux = psS.tile([128, 512], F32, tag="scr", bufs=2)
                nc.tensor.matmul(aux[:], perm_sb[:], raw[:],
                                 start=True, stop=True)
                swp = sbA.tile([128, 512], BF16, tag="swp", bufs=3)
                nc.vector.tensor_mul(swp[:], aux[:], sin_sb[:, sl])
                # late subtiles' cos-mul rides Pool: their rope runs during
                # the elementwise-bound late rows where DVE is the scarce
                # engine (early rows keep the short DVE chain)
                if s >= 3:
                    nc.gpsimd.tensor_mul(dst[:, sl], raw[:], cos_sb[:, sl])
                else:
                    nc.vector.tensor_mul(dst[:, sl], raw[:], cos_sb[:, sl])
                if s <= 2:
                    nc.vector.tensor_add(dst[:, sl], dst[:, sl], swp[:])
                else:
                    nc.gpsimd.tensor_add(dst[:, sl], dst[:, sl], swp[:])
                yield
            # V projected directly in [seq, dh] orientation (x-block
            # stationary, Wv^T chunk moving): no PE transposes and a single
            # strided copy into v_sb's [key, dh|ones] layout.
            psv = psS.tile([128, 512], F32, tag="scr", bufs=2)
            for blk in range(4):
                for ch in range(ND):
                    nc.tensor.matmul(
                        psv[:, bass.ts(blk, 128)],
                        xt_t[:, bass.ds(ch * 512 + blk * 128, 128)],
                        wv_sb[:, bass.ts(ch, 128)],
                        start=(blk == 0 and ch == 0), stop=ch == ND - 1,
                        skip_group_check=True)
                if blk == 1:
                    yield
            src = psv[:].rearrange("p (b h c) -> p b h c", b=4, h=2)
            vdst = v_sb[:, bass.ds(130 * 4 * s, 130 * 4)].rearrange(
                "p (b h c) -> p b h c", b=4, c=65)[:, :, :, 0:64]
            nc.scalar.copy(vdst, src)
            yield

        state = {}  # qb -> (O_sb, invs) for deferred norm/o_proj

        def norm_oproj_units(qb, tail=False):
            """Normalize + o_proj + store for row qb, one unit per 128-query
            slab so it can be woven into a later attention row."""
            O_t, invs = state.pop(qb)
            for i in range(8):
                nc.gpsimd.tensor_scalar_mul(
                    O_t[:, bass.ds(64 * i, 64)],
                    O_t[:, bass.ds(64 * i, 64)],
                    invs[:, i:i + 1])
            yield
            yield
            yield
            trp_t = psS.tile([128, 512], F32, tag="scr", bufs=2)
            trpb = trp_t[:].bitcast(BF16)
            for qs in range(4):
                nc.tensor.matmul(trpb[:, bass.ts(qs, 128)],
                                 O_t[:, bass.ts(qs, 128)], idb_sb[:],
                                 is_transpose=True, skip_group_check=True)
            ot_t = sbC.tile([128, 512], BF16, tag="ot", bufs=2)
            nc.vector.tensor_copy(ot_t[:], trpb[:, 0:512])
            yield
            for qs in range(4):
                lc = 4 * qb + qs
                ob = sbC.tile([128, 1024], BF16, tag="ob", bufs=5)
                for n in range(2):
                    op = psS.tile([128, 512], F32, tag="scr", bufs=2)
                    nc.tensor.matmul(op[:], ot_t[:, bass.ts(qs, 128)],
                                     wo_sb[:, bass.ts(n, 512)],
                                     start=True, stop=True)
                    # PSUM->SBUF conversion split ACT/DVE so neither
                    # serializes (Pool can't read PSUM)
                    if n == 0:
                        nc.scalar.copy(ob[:, bass.ts(n, 512)], op[:])
                    else:
                        nc.vector.tensor_copy(ob[:, bass.ts(n, 512)], op[:])
                nc.sync.dma_start(partial[bass.ts(lc, 128), :], ob[:])
                yield

        # PSUM accumulate-group state is per bank: region 7 would cross
        # the 2048B bank boundary at col 455, so it lives at col 512.
        PVC = [65 * i for i in range(7)] + [512]

        def emit_exp(s01, p01, kb, r, split=False, act23=False):
            """Exp of one kb's scores on a single engine, alternating ACT
            (native exp) and DVE (bf16 Schraudolph) per kb: one full-width
            instruction amortizes the PSUM access latency, and with two
            engines each handling every other kb the pipeline pace is
            ~max_instr/2 per kb, under the per-kb PE time. Pool can't read
            PSUM, so it takes no exp share. split=True halves the latency
            (h0 on ACT, h1 on DVE) for the row-end kbs that gate the
            softmax-sum drain."""
            if split and r < 0:
                nc.scalar.activation(p01[:, 0:512], s01[:, 0:512], ACT_EXP,
                                     scale=SCALE)
                nc.vector.tensor_scalar(
                    p01[:].bitcast(I16)[:, 512:1024], s01[:, 512:1024],
                    SCH_A, SCH_B,
                    mybir.AluOpType.mult, mybir.AluOpType.add)
                return
            if split and r >= 0:
                c0 = 128 * r
                nc.scalar.activation(p01[:, c0:512], s01[:, c0:512],
                                     ACT_EXP, scale=SCALE)
                nc.vector.tensor_scalar(
                    p01[:].bitcast(I16)[:, 512 + c0:1024],
                    s01[:, 512 + c0:1024], SCH_A, SCH_B,
                    mybir.AluOpType.mult, mybir.AluOpType.add)
                return
            if (kb % 3 != 1) if act23 else (kb % 2 == 0 or kb == 1):
                if r < 0:
                    nc.scalar.activation(p01[:], s01[:], ACT_EXP, scale=SCALE)
                else:
                    c0 = 128 * r
                    sin_ = s01[:].rearrange(
                        "p (h c) -> p h c", h=2)[:, :, c0:512]
                    pout = p01[:].rearrange(
                        "p (h c) -> p h c", h=2)[:, :, c0:512]
                    nc.scalar.activation(pout, sin_, ACT_EXP, scale=SCALE)
            else:
                if r < 0:
                    nc.vector.tensor_scalar(
                        p01[:].bitcast(I16), s01[:], SCH_A, SCH_B,
                        mybir.AluOpType.mult, mybir.AluOpType.add)
                else:
                    c0 = 128 * r
                    sin_ = s01[:].rearrange(
                        "p (h c) -> p h c", h=2)[:, :, c0:512]
                    pout = p01[:].bitcast(I16).rearrange(
                        "p (h c) -> p h c", h=2)[:, :, c0:512]
                    nc.vector.tensor_scalar(
                        pout, sin_, SCH_A, SCH_B,
                        mybir.AluOpType.mult, mybir.AluOpType.add)

        def attention(qb, weave=()):
            """Row qb. PV for kb trails QK by 2 key blocks so the exp of kb
            (split across ACT/DVE/Pool) finishes well before PE consumes it.
            `weave` units (next row's projections, previous row's o_proj)
            are drained evenly across the kb loop to fill PE/elementwise
            gaps."""
            weave = iter(weave)
            qsl0 = 512 * qb
            pvacc = psB.tile([128, 577], F32, tag="pv", bufs=1)
            nkb = 4 * qb + 4
            pend = []            # [(kb, p01)] awaiting PV

            def emit_pv(kb, p01):
                r = kb - 4 * qb
                for qs in range(max(0, r), 4):
                    for h in range(2):
                        i = 2 * qs + h
                        nc.tensor.matmul(
                            pvacc[:, bass.ds(PVC[i], 65)],
                            p01[:, bass.ds(512 * h + 128 * qs, 128)],
                            v_sb[:, bass.ds(130 * kb + 65 * h, 65)],
                            start=(kb == 0 and i in (0, 7)),
                            stop=kb == 4 * qb + qs,
                            skip_group_check=True)

            for kb in range(nkb):
                r = kb - 4 * qb
                c0 = 128 * r if r > 0 else 0
                W = 512 - c0
                ksl = bass.ts(kb, 128)
                qsl = bass.ds(qsl0 + c0, W)
                last = qb == NQB - 1
                split = kb >= nkb - (4 if last else 2)
                pe_mask = r >= 0 and last
                s01 = psB.tile([128, 1024], F32, tag="sc", bufs=2)
                nc.tensor.matmul(s01[:, c0:512], kT[0:64, ksl],
                                 qT[0:64, qsl], start=True, stop=True)
                if pe_mask:
                    # last row's diagonal gates the output tail: mask h0 on
                    # PE (-3e4 bias -> ACT exp gives exact zeros). h1 goes
                    # through DVE Schraudolph, which wraps on that range, so
                    # it keeps the tril multiply.
                    nc.tensor.matmul(s01[:, c0:c0 + 128], trib_sb[:],
                                     idb_sb[:], start=False, stop=True,
                                     skip_group_check=True)
                nc.tensor.matmul(s01[:, 512 + c0:1024], kT[64:128, ksl],
                                 qT[64:128, qsl], start=True, stop=True)
                p01 = sbB.tile([128, 1024], BF16, tag="p01", bufs=10)
                emit_exp(s01, p01, kb, r, split=split,
                         act23=qb in (5, 6))
                if r >= 0:
                    # diagonal masks pace the row tail: DVE's 133ns beats
                    # Pool's 349ns there
                    for h in range(2):
                        if h == 0 and pe_mask:
                            continue
                        msl = bass.ds(512 * h + c0, 128)
                        nc.vector.tensor_mul(p01[:, msl], p01[:, msl],
                                             tril_sb[:])
                pend.append((kb, p01))
                if len(pend) > (6 if qb in (5, 6) else 3):
                    emit_pv(*pend.pop(0))
                next(weave, None)
            if qb == NQB - 1:
                for _ in weave:
                    pass
                # last row: pipeline the per-qs norm/o_proj chains against
                # the trailing PV drains; (qs,h) accumulation stops at
                # kb = 4*qb + qs, so qs is final once that kb is PV'd.
                def tail_qs(qs):
                    invs = sbB.tile([128, 2], F32, tag="invq", bufs=4)
                    O_q = sbB.tile([128, 128], BF16, tag="osq", bufs=4)
                    for j, i in enumerate((2 * qs, 2 * qs + 1)):
                        sc_ = 512 if i == 7 else PVC[i]
                        nc.vector.reciprocal(invs[:, j:j + 1],
                                             pvacc[:, sc_ + 64:sc_ + 65])
                        nc.scalar.mul(O_q[:, bass.ts(j, 64)],
                                      pvacc[:, sc_:sc_ + 64],
                                      invs[:, j:j + 1])
                    trp_t = psS.tile([128, 512], F32, tag="scr", bufs=2)
                    trpb = trp_t[:].bitcast(BF16)
                    nc.tensor.matmul(trpb[:, 0:128], O_q[:], idb_sb[:],
                                     is_transpose=True,
                                     skip_group_check=True)
                    ot_q = sbC.tile([128, 128], BF16, tag="otq", bufs=3)
                    nc.vector.tensor_copy(ot_q[:], trpb[:, 0:128])
                    ob = sbC.tile([128, 1024], BF16, tag="ob", bufs=5)
                    for n in range(2):
                        op = psS.tile([128, 512], F32, tag="scr", bufs=2)
                        nc.tensor.matmul(op[:], ot_q[:],
                                         wo_sb[:, bass.ts(n, 512)],
                                         start=True, stop=True)
                        if n == 0:
                            nc.scalar.copy(ob[:, bass.ts(n, 512)], op[:])
                        else:
                            nc.vector.tensor_copy(ob[:, bass.ts(n, 512)],
                                                  op[:])
                    nc.sync.dma_start(
                        partial[bass.ts(4 * qb + qs, 128), :], ob[:])

                tail_qs(0)
                done = 1
                for it in pend:
                    emit_pv(*it)
                    if done <= 3:
                        tail_qs(done)
                        done += 1
                while done <= 3:
                    tail_qs(done)
                    done += 1
                return
            for it in pend:
                emit_pv(*it)
            # free pvacc quickly: reciprocal of sums + copy out
            invs = sbB.tile([128, 8], F32, tag="invs", bufs=6)
            sums7 = pvacc[:, 0:455].rearrange(
                "p (i c) -> p i c", c=65)[:, :, 64]
            nc.vector.reciprocal(invs[:, 0:7], sums7)
            nc.vector.reciprocal(invs[:, 7:8], pvacc[:, 576:577])
            O_t = sbB.tile([128, 512], BF16, tag="osb", bufs=6)
            psrc7 = pvacc[:, 0:455].rearrange(
                "p (i c) -> p i c", c=65)[:, :, 0:64]
            nc.vector.tensor_copy(
                O_t[:, 0:448].rearrange("p (i c) -> p i c", c=64), psrc7)
            nc.vector.tensor_copy(O_t[:, 448:512], pvacc[:, 512:576])
            state[qb] = (O_t, invs)
            for _ in weave:      # drain anything the kb loop didn't cover
                pass

        # Interleave: attention row qb needs keys 0..512*(qb+1) = subtiles
        # 0..qb, so row s can run right after phase_a(s). Projections run
        # two rows ahead (hides the full RoPE chain) and the deferred
        # norm/o_proj units are pushed into the late, gap-rich rows.
        import itertools
        for _ in phase_a_units(0):
            pass
        norm_sched = {s: [s - 1] for s in range(1, 8)}
        for s in range(NSB):
            w = []
            if s + 1 < NSB:
                w.append(phase_a_units(s + 1))
            for nb in norm_sched.get(s, []):
                w.append(norm_oproj_units(nb))
            attention(s, weave=itertools.chain(*w))
        # the last row normalizes itself via the per-qs tail pipeline


def build(L_=L, debug=False):
    nc = bacc.Bacc("TRN2", target_bir_lowering=False, debug=debug,
                   enable_asserts=False)
    aps = {}
    NSB = L_ // 512
    aps["xt"] = nc.dram_tensor("xt", [128, NSB * 4096], BF16,
                               kind="ExternalInput").ap()
    aps["wq"] = nc.dram_tensor("wq", [128, D], BF16,
                               kind="ExternalInput").ap()
    aps["wkv"] = nc.dram_tensor("wkv", [128, 2 * D], BF16,
                                kind="ExternalInput").ap()
    aps["wo"] = nc.dram_tensor("wo", [128, D], BF16,
                               kind="ExternalInput").ap()
    aps["cs"] = nc.dram_tensor("cs", [128, 2 * L_], BF16,
                               kind="ExternalInput").ap()
    aps["tip"] = nc.dram_tensor("tip", [128, 512], BF16,
                                kind="ExternalInput").ap()
    aps["partial"] = nc.dram_tensor("partial", [L_, D], BF16,
                                    kind="ExternalOutput").ap()

    with tile.TileContext(nc) as tc:
        emit(nc, tc, aps, L_)
    nc.compile()
    return nc, aps


def make_in_maps(x, Wq, Wk, Wv, Wo, L_=L):
    xr = _layout_x(x, L_)
    consts = _host_consts(L_)
    in_maps = []
    for c in range(N_CORES):
        wq, wkv, wo = _core_weights(c, Wq, Wk, Wv, Wo)
        m = {"xt": xr, "wq": wq, "wkv": wkv, "wo": wo}
        m.update(consts)
        in_maps.append(m)
    return in_maps


_CACHE = {}


def _run(inputs, trace=False, **kw):
    if trace:
        os.environ.pop("BASS_NEVER_TRACE", None)
    x = np.asarray(inputs["x"], np.float32)
    Wq = np.asarray(inputs["Wq"], np.float32)
    Wk = np.asarray(inputs["Wk"], np.float32)
    Wv = np.asarray(inputs["Wv"], np.float32)
    Wo = np.asarray(inputs["Wo"], np.float32)
    if "nc" not in _CACHE:
        _CACHE["nc"] = build()[0]
    nc = _CACHE["nc"]
    in_maps = make_in_maps(x, Wq, Wk, Wv, Wo)
    res = run_bass_kernel_spmd(nc, in_maps, core_ids=list(range(N_CORES)),
                               trace=trace, **kw)
    acc = np.zeros((L, D), np.float64)
    for r in res.results:
        acc += r["partial"].astype(np.float64)
    out = acc.astype(np.float32).reshape(B, L, D)
    return out, res


def kernel(**inputs):
    out, _ = _run(inputs)
    return out



# revision 88
# speedup vs baseline: 1.0120x; 1.0120x over previous
"""Trainium2 Bass kernel for nn_BloqueAttn: causal RoPE attention, 16 heads,
head-sharded (tensor-parallel) across 8 NeuronCores, o_proj row-sharded with
host-side all-reduce of the partials.

v2: bf16 datapath, query-on-partition PV (65-wide moving operand), PE
perm-matmul RoPE swap, mask-by-multiply on DVE, per-partition softmax
normalization, batched DMAs with host-side pre-layout.

Self-contained: hardcodes shapes B=1, L=4096, D=1024, H=16, DH=64, 8 cores.
"""
import os

os.environ.setdefault("BASS_NEVER_TRACE", "1")

import numpy as np
import ml_dtypes

import concourse.bass as bass
import concourse.bacc as bacc
import concourse.mybir as mybir
import concourse.tile as tile
from concourse.bass_utils import run_bass_kernel_spmd

F32 = mybir.dt.float32
BF16 = mybir.dt.bfloat16
I16 = mybir.dt.int16

B, L, D = 1, 4096, 1024
H, DH = 16, 64
BASE = 10000.0
N_CORES = 8
HPC = H // N_CORES          # heads per core = 2
DH2 = HPC * DH              # packed head dim = 128
SCALE = DH ** -0.5          # 0.125

# Schraudolph-style exp in bf16 bits: bf16(e^(x*SCALE)) ~= bits of
# int16(A*x + B) with A = SCALE * 2^7 / ln2, B = 127*2^7 - 7.41 (minimax).
SCH_A = SCALE * 128.0 / np.log(2.0)
SCH_B = 16256.0 - 7.41


# ---------------------------------------------------------------- host helpers

def _rope_tables(L_, dh):
    inv_freq = 1.0 / (BASE ** (np.arange(0, dh, 2, dtype=np.float32) / dh))
    freqs = np.outer(np.arange(L_, dtype=np.float32), inv_freq)  # [L, 32]
    return np.cos(freqs).astype(np.float32), np.sin(freqs).astype(np.float32)


def _host_consts(L_):
    cos, sin = _rope_tables(L_, DH)          # [L, 32]
    cosT, sinT = cos.T.copy(), sin.T.copy()  # [32, L]
    cos_stack = np.concatenate([cosT, cosT, cosT, cosT], 0)          # [128, L]
    sin_signed = np.concatenate([-sinT, sinT, -sinT, sinT], 0)       # [128, L]

    # 0/1 causal keep-mask within a 128x128 diagonal block:
    # key j visible to query c iff j <= c.
    j = np.arange(128)[:, None]
    c = np.arange(128)[None, :]
    tril01 = (j <= c).astype(np.float32)                             # [128,128]

    ident = np.eye(128, dtype=np.float32)
    # 32-row block swap permutation: out[i] = in[sigma(i)],
    # sigma = [32..63, 0..31, 96..127, 64..95]
    sigma = np.concatenate([np.arange(32, 64), np.arange(0, 32),
                            np.arange(96, 128), np.arange(64, 96)])
    pmat = np.zeros((128, 128), np.float32)
    pmat[sigma, np.arange(128)] = 1.0        # out = pmat.T @ in
    cs = np.concatenate([cos_stack, sin_signed], 1)          # [128, 2L]
    # trib: lhsT for the PE mask-bias matmul (identity moving):
    # out[m, n] = trib[n, m] = -29952 where m > n
    trib = np.triu(np.full((128, 128), -29952.0, np.float32), 1)
    tip = np.concatenate([tril01, ident, pmat, trib], 1)     # [128, 512]
    return {
        "cs": cs.astype(ml_dtypes.bfloat16),
        "tip": tip.astype(ml_dtypes.bfloat16),
    }


def _chunk_major(wT):
    """[D, 128] -> [128, D] with 128-row chunks laid side by side."""
    ndc = wT.shape[0] // 128
    return np.ascontiguousarray(
        wT.reshape(ndc, 128, 128).transpose(1, 0, 2).reshape(128, ndc * 128))


def _core_weights(core, Wq, Wk, Wv, Wo):
    """Per-core weight slices, bf16, chunk-major; RoPE even/odd permutation
    applied to Wq/Wk rows."""
    perm = np.concatenate([np.arange(0, DH, 2), np.arange(1, DH, 2)])  # [64]
    rows_p, rows = [], []
    for hh in (HPC * core, HPC * core + 1):
        rows_p.append(DH * hh + perm)
        rows.append(DH * hh + np.arange(DH))
    rows_p = np.concatenate(rows_p)
    rows = np.concatenate(rows)
    wq = _chunk_major(Wq[rows_p, :].T).astype(ml_dtypes.bfloat16)  # [128, 1024]
    wk = _chunk_major(Wk[rows_p, :].T)
    wv = _chunk_major(Wv[rows, :].T)
    woC = np.ascontiguousarray(Wo[:, DH2 * core: DH2 * (core + 1)].T)
    wkv = np.concatenate([wk, wv], 1).astype(ml_dtypes.bfloat16)
    return wq, wkv, woC.astype(ml_dtypes.bfloat16)


def _layout_x(x, L_):
    """x [B,L,D] -> [128, 8*4096] bf16, 512-col subtile-major:
    xr[p, s*4096 + ch*512 + c] = x[s*512+c, ch*128+p]."""
    xT = np.ascontiguousarray(x.reshape(L_, D).T)        # [D, L]
    ns = L_ // 512
    xr = xT.reshape(8, 128, ns, 512).transpose(1, 2, 0, 3)
    return np.ascontiguousarray(xr.reshape(128, ns * 4096)).astype(
        ml_dtypes.bfloat16)


# ---------------------------------------------------------------- device emit

def emit(nc, tc, aps, L_):
    NSB = L_ // 512           # 512-col subtiles (8) == query blocks
    NQB = L_ // 512
    NKB = L_ // 128           # key blocks (32)
    ND = D // 128             # D chunks (8)

    xt = aps["xt"]
    partial = aps["partial"]
    ACT_EXP = mybir.ActivationFunctionType.Exp

    with tc.tile_pool(name="persist", bufs=1) as pp, \
         tc.tile_pool(name="psB", bufs=1, space="PSUM") as psB, \
         tc.tile_pool(name="psS", bufs=1, space="PSUM") as psS, \
         tc.tile_pool(name="sbC", bufs=1) as sbC, \
         tc.tile_pool(name="sbB", bufs=1) as sbB, \
         tc.tile_pool(name="sbA", bufs=1) as sbA:
        wq_sb = pp.tile([128, D], BF16)
        wkvo_sb = pp.tile([128, 3 * D], BF16)
        wk_sb = wkvo_sb[:, 0:D]
        wv_sb = wkvo_sb[:, D:2 * D]
        wo_sb = wkvo_sb[:, 2 * D:3 * D]
        wkv_view = wkvo_sb[:, 0:2 * D]
        cs_sb = pp.tile([128, 2 * L_], BF16)
        cos_sb = cs_sb[:, 0:L_]
        sin_sb = cs_sb[:, L_:2 * L_]
        tip_sb = pp.tile([128, 512], BF16)
        tril_sb = tip_sb[:, 0:128]
        idb_sb = tip_sb[:, 128:256]
        perm_sb = tip_sb[:, 256:384]
        trib_sb = tip_sb[:, 384:512]
        qT = pp.tile([128, L_], BF16)
        kT = pp.tile([128, L_], BF16)
        v_sb = pp.tile([128, NKB * 130], BF16)
        # PE pstate warmup: the cost model ramps the PE clock over 3us from
        # the first matmul; dummy matmuls during the initial DMA wait start
        # the ramp early so real work runs at mid/full speed sooner. They
        # also cover the window until xt chunk 7 lands for the s=0 q-proj.
        wup = pp.tile([128, 256], BF16)
        nc.gpsimd.memset(wup[:], 0.0)
        nc.sync.dma_start(wq_sb[:], aps["wq"][:])
        ones_cols = v_sb[:].rearrange("p (kb h c) -> p kb h c",
                                      h=2, c=65)[:, :, :, 64:65]
        nc.gpsimd.memset(ones_cols, 1.0)  # ones columns for the sum trick
        for _ in range(12):
            wps = psS.tile([128, 512], F32, tag="scr", bufs=2)
            nc.tensor.matmul(wps[:, 0:256], wup[:, 0:128], wup[:],
                             start=True, stop=True)

        def phase_a_units(s):
            """Projections + RoPE + V transpose for L-subtile s, as a
            generator: each next() emits one schedulable unit so the caller
            can weave these between attention key blocks."""
            sl = bass.ds(512 * s, 512)
            xt_t = sbA.tile([128, 4096], BF16, tag="xt", bufs=3)
            if s == 0:
                # batched input streaming: HWDGE costs ~625ns per DMA, so
                # few large DMAs beat many small ones
                nc.sync.dma_start(xt_t[:, 0:2048], xt[:, 0:2048])
                nc.sync.dma_start(xt_t[:, 2048:4096], xt[:, 2048:4096])
                nc.sync.dma_start(wkv_view, aps["wkv"][:])
                nc.sync.dma_start(tip_sb[:], aps["tip"][:])
                # (wq DMA is issued before the warmup matmuls)
                # only the first 512-col slice of cos/sin is needed for s=0;
                # the rest streams in behind xt(1) to unblock it
                cs_head_d = aps["cs"][:].rearrange(
                    "p (t c) -> p t c", t=2)[:, :, 0:512]
                cs_head_s = cs_sb[:].rearrange(
                    "p (t c) -> p t c", t=2)[:, :, 0:512]
                nc.sync.dma_start(cs_head_s, cs_head_d)
            else:
                nc.sync.dma_start(xt_t[:], xt[:, bass.ts(s, 4096)])
                if s == 1:
                    nc.sync.dma_start(wo_sb[:], aps["wo"][:])
                    cs_rest_d = aps["cs"][:].rearrange(
                        "p (t c) -> p t c", t=2)[:, :, 512:L_]
                    cs_rest_s = cs_sb[:].rearrange(
                        "p (t c) -> p t c", t=2)[:, :, 512:L_]
                    nc.sync.dma_start(cs_rest_s, cs_rest_d)
            yield
            raws = {}
            # q/k projections first so their RoPE (DVE) overlaps the
            # V projection + transposes (PE) and attention(s) starts clean.
            for name, wsb in (("q", wq_sb), ("k", wk_sb)):
                ps = psS.tile([128, 512], F32, tag="scr", bufs=2)
                for ch in range(ND):
                    nc.tensor.matmul(ps[:], wsb[:, bass.ts(ch, 128)],
                                     xt_t[:, bass.ts(ch, 512)],
                                     start=ch == 0, stop=ch == ND - 1)
                    if ch == 3:
                        yield
                raw = sbA.tile([128, 512], BF16, tag=f"raw{name}", bufs=4)
                nc.scalar.copy(raw[:], ps[:])
                raws[name] = raw
                yield
            # RoPE: rot = raw*cos + perm(raw)*sin_signed. The final add is
            # SBUF-only so it rides Pool, freeing DVE for exp conversions.
            for name, dst in (("q", qT), ("k", kT)):
                raw = raws[name]
                aux = psS.tile([128, 512], F32, tag="scr", bufs=2)
                nc.tensor.matmul(aux[:], perm_sb[:], raw[:],
                                 start=True, stop=True)
                swp = sbA.tile([128, 512], BF16, tag="swp", bufs=3)
                nc.vector.tensor_mul(swp[:], aux[:], sin_sb[:, sl])
                # late subtiles' cos-mul rides Pool: their rope runs during
                # the elementwise-bound late rows where DVE is the scarce
                # engine (early rows keep the short DVE chain)
                if s >= 3:
                    nc.gpsimd.tensor_mul(dst[:, sl], raw[:], cos_sb[:, sl])
                else:
                    nc.vector.tensor_mul(dst[:, sl], raw[:], cos_sb[:, sl])
                if s <= 2:
                    nc.vector.tensor_add(dst[:, sl], dst[:, sl], swp[:])
                else:
                    nc.gpsimd.tensor_add(dst[:, sl], dst[:, sl], swp[:])
                yield
            # V projected directly in [seq, dh] orientation (x-block
            # stationary, Wv^T chunk moving): no PE transposes and a single
            # strided copy into v_sb's [key, dh|ones] layout.
            psv = psS.tile([128, 512], F32, tag="scr", bufs=2)
            for blk in range(4):
                for ch in range(ND):
                    nc.tensor.matmul(
                        psv[:, bass.ts(blk, 128)],
                        xt_t[:, bass.ds(ch * 512 + blk * 128, 128)],
                        wv_sb[:, bass.ts(ch, 128)],
                        start=(blk == 0 and ch == 0), stop=ch == ND - 1,
                        skip_group_check=True)
                if blk == 1:
                    yield
            src = psv[:].rearrange("p (b h c) -> p b h c", b=4, h=2)
            vdst = v_sb[:, bass.ds(130 * 4 * s, 130 * 4)].rearrange(
                "p (b h c) -> p b h c", b=4, c=65)[:, :, :, 0:64]
            nc.scalar.copy(vdst, src)
            yield

        state = {}  # qb -> (O_sb, invs) for deferred norm/o_proj

        def norm_oproj_units(qb, tail=False):
            """Normalize + o_proj + store for row qb, one unit per 128-query
            slab so it can be woven into a later attention row."""
            O_t, invs = state.pop(qb)
            for i in range(8):
                nc.gpsimd.tensor_scalar_mul(
                    O_t[:, bass.ds(64 * i, 64)],
                    O_t[:, bass.ds(64 * i, 64)],
                    invs[:, i:i + 1])
            yield
            yield
            yield
            trp_t = psS.tile([128, 512], F32, tag="scr", bufs=2)
            trpb = trp_t[:].bitcast(BF16)
            for qs in range(4):
                nc.tensor.matmul(trpb[:, bass.ts(qs, 128)],
                                 O_t[:, bass.ts(qs, 128)], idb_sb[:],
                                 is_transpose=True, skip_group_check=True)
            ot_t = sbC.tile([128, 512], BF16, tag="ot", bufs=2)
            nc.vector.tensor_copy(ot_t[:], trpb[:, 0:512])
            yield
            for qs in range(4):
                lc = 4 * qb + qs
                ob = sbC.tile([128, 1024], BF16, tag="ob", bufs=5)
                for n in range(2):
                    op = psS.tile([128, 512], F32, tag="scr", bufs=2)
                    nc.tensor.matmul(op[:], ot_t[:, bass.ts(qs, 128)],
                                     wo_sb[:, bass.ts(n, 512)],
                                     start=True, stop=True)
                    # PSUM->SBUF conversion split ACT/DVE so neither
                    # serializes (Pool can't read PSUM)
                    if n == 0:
                        nc.scalar.copy(ob[:, bass.ts(n, 512)], op[:])
                    else:
                        nc.vector.tensor_copy(ob[:, bass.ts(n, 512)], op[:])
                nc.sync.dma_start(partial[bass.ts(lc, 128), :], ob[:])
                yield

        # PSUM accumulate-group state is per bank: region 7 would cross
        # the 2048B bank boundary at col 455, so it lives at col 512.
        PVC = [65 * i for i in range(7)] + [512]

        def emit_exp(s01, p01, kb, r, split=False):
            """Exp of one kb's scores on a single engine, alternating ACT
            (native exp) and DVE (bf16 Schraudolph) per kb: one full-width
            instruction amortizes the PSUM access latency, and with two
            engines each handling every other kb the pipeline pace is
            ~max_instr/2 per kb, under the per-kb PE time. Pool can't read
            PSUM, so it takes no exp share. split=True halves the latency
            (h0 on ACT, h1 on DVE) for the row-end kbs that gate the
            softmax-sum drain."""
            if split and r < 0:
                nc.scalar.activation(p01[:, 0:512], s01[:, 0:512], ACT_EXP,
                                     scale=SCALE)
                nc.vector.tensor_scalar(
                    p01[:].bitcast(I16)[:, 512:1024], s01[:, 512:1024],
                    SCH_A, SCH_B,
                    mybir.AluOpType.mult, mybir.AluOpType.add)
                return
            if split and r >= 0:
                c0 = 128 * r
                nc.scalar.activation(p01[:, c0:512], s01[:, c0:512],
                                     ACT_EXP, scale=SCALE)
                nc.vector.tensor_scalar(
                    p01[:].bitcast(I16)[:, 512 + c0:1024],
                    s01[:, 512 + c0:1024], SCH_A, SCH_B,
                    mybir.AluOpType.mult, mybir.AluOpType.add)
                return
            if kb % 2 == 0 or kb == 1:
                if r < 0:
                    nc.scalar.activation(p01[:], s01[:], ACT_EXP, scale=SCALE)
                else:
                    c0 = 128 * r
                    sin_ = s01[:].rearrange(
                        "p (h c) -> p h c", h=2)[:, :, c0:512]
                    pout = p01[:].rearrange(
                        "p (h c) -> p h c", h=2)[:, :, c0:512]
                    nc.scalar.activation(pout, sin_, ACT_EXP, scale=SCALE)
            else:
                if r < 0:
                    nc.vector.tensor_scalar(
                        p01[:].bitcast(I16), s01[:], SCH_A, SCH_B,
                        mybir.AluOpType.mult, mybir.AluOpType.add)
                else:
                    c0 = 128 * r
                    sin_ = s01[:].rearrange(
                        "p (h c) -> p h c", h=2)[:, :, c0:512]
                    pout = p01[:].bitcast(I16).rearrange(
                        "p (h c) -> p h c", h=2)[:, :, c0:512]
                    nc.vector.tensor_scalar(
                        pout, sin_, SCH_A, SCH_B,
                        mybir.AluOpType.mult, mybir.AluOpType.add)

        def attention(qb, weave=()):
            """Row qb. PV for kb trails QK by 2 key blocks so the exp of kb
            (split across ACT/DVE/Pool) finishes well before PE consumes it.
            `weave` units (next row's projections, previous row's o_proj)
            are drained evenly across the kb loop to fill PE/elementwise
            gaps."""
            weave = iter(weave)
            qsl0 = 512 * qb
            pvacc = psB.tile([128, 577], F32, tag="pv", bufs=1)
            nkb = 4 * qb + 4
            pend = []            # [(kb, p01)] awaiting PV

            def emit_pv(kb, p01):
                r = kb - 4 * qb
                for qs in range(max(0, r), 4):
                    for h in range(2):
                        i = 2 * qs + h
                        nc.tensor.matmul(
                            pvacc[:, bass.ds(PVC[i], 65)],
                            p01[:, bass.ds(512 * h + 128 * qs, 128)],
                            v_sb[:, bass.ds(130 * kb + 65 * h, 65)],
                            start=(kb == 0 and i in (0, 7)),
                            stop=kb == 4 * qb + qs,
                            skip_group_check=True)

            for kb in range(nkb):
                r = kb - 4 * qb
                c0 = 128 * r if r > 0 else 0
                W = 512 - c0
                ksl = bass.ts(kb, 128)
                qsl = bass.ds(qsl0 + c0, W)
                last = qb == NQB - 1
                split = kb >= nkb - (4 if last else 2)
                pe_mask = r >= 0 and last
                s01 = psB.tile([128, 1024], F32, tag="sc", bufs=2)
                nc.tensor.matmul(s01[:, c0:512], kT[0:64, ksl],
                                 qT[0:64, qsl], start=True, stop=True)
                if pe_mask:
                    # last row's diagonal gates the output tail: mask h0 on
                    # PE (-3e4 bias -> ACT exp gives exact zeros). h1 goes
                    # through DVE Schraudolph, which wraps on that range, so
                    # it keeps the tril multiply.
                    nc.tensor.matmul(s01[:, c0:c0 + 128], trib_sb[:],
                                     idb_sb[:], start=False, stop=True,
                                     skip_group_check=True)
                nc.tensor.matmul(s01[:, 512 + c0:1024], kT[64:128, ksl],
                                 qT[64:128, qsl], start=True, stop=True)
                p01 = sbB.tile([128, 1024], BF16, tag="p01", bufs=10)
                emit_exp(s01, p01, kb, r, split=split)
                if r >= 0:
                    # diagonal masks pace the row tail: DVE's 133ns beats
                    # Pool's 349ns there
                    for h in range(2):
                        if h == 0 and pe_mask:
                            continue
                        msl = bass.ds(512 * h + c0, 128)
                        nc.vector.tensor_mul(p01[:, msl], p01[:, msl],
                                             tril_sb[:])
                pend.append((kb, p01))
                if len(pend) > (6 if qb in (5, 6) else 3):
                    emit_pv(*pend.pop(0))
                next(weave, None)
            if qb == NQB - 1:
                for _ in weave:
                    pass
                # last row: pipeline the per-qs norm/o_proj chains against
                # the trailing PV drains; (qs,h) accumulation stops at
                # kb = 4*qb + qs, so qs is final once that kb is PV'd.
                def tail_qs(qs):
                    invs = sbB.tile([128, 2], F32, tag="invq", bufs=4)
                    O_q = sbB.tile([128, 128], BF16, tag="osq", bufs=4)
                    for j, i in enumerate((2 * qs, 2 * qs + 1)):
                        sc_ = 512 if i == 7 else PVC[i]
                        nc.vector.reciprocal(invs[:, j:j + 1],
                                             pvacc[:, sc_ + 64:sc_ + 65])
                        nc.scalar.mul(O_q[:, bass.ts(j, 64)],
                                      pvacc[:, sc_:sc_ + 64],
                                      invs[:, j:j + 1])
                    trp_t = psS.tile([128, 512], F32, tag="scr", bufs=2)
                    trpb = trp_t[:].bitcast(BF16)
                    nc.tensor.matmul(trpb[:, 0:128], O_q[:], idb_sb[:],
                                     is_transpose=True,
                                     skip_group_check=True)
                    ot_q = sbC.tile([128, 128], BF16, tag="otq", bufs=3)
                    nc.vector.tensor_copy(ot_q[:], trpb[:, 0:128])
                    ob = sbC.tile([128, 1024], BF16, tag="ob", bufs=5)
                    for n in range(2):
                        op = psS.tile([128, 512], F32, tag="scr", bufs=2)
                        nc.tensor.matmul(op[:], ot_q[:],
                                         wo_sb[:, bass.ts(n, 512)],
                                         start=True, stop=True)
                        if n == 0:
                            nc.scalar.copy(ob[:, bass.ts(n, 512)], op[:])
                        else:
                            nc.vector.tensor_copy(ob[:, bass.ts(n, 512)],
                                                  op[:])
                    nc.sync.dma_start(
                        partial[bass.ts(4 * qb + qs, 128), :], ob[:])

                tail_qs(0)
                done = 1
                for it in pend:
                    emit_pv(*it)
                    if done <= 3:
                        tail_qs(done)
                        done += 1
                while done <= 3:
                    tail_qs(done)
                    done += 1
                return
            for it in pend:
                emit_pv(*it)
            # free pvacc quickly: reciprocal of sums + copy out
            invs = sbB.tile([128, 8], F32, tag="invs", bufs=6)
            sums7 = pvacc[:, 0:455].rearrange(
                "p (i c) -> p i c", c=65)[:, :, 64]
            nc.vector.reciprocal(invs[:, 0:7], sums7)
            nc.vector.reciprocal(invs[:, 7:8], pvacc[:, 576:577])
            O_t = sbB.tile([128, 512], BF16, tag="osb", bufs=6)
            psrc7 = pvacc[:, 0:455].rearrange(
                "p (i c) -> p i c", c=65)[:, :, 0:64]
            nc.vector.tensor_copy(
                O_t[:, 0:448].rearrange("p (i c) -> p i c", c=64), psrc7)
            nc.vector.tensor_copy(O_t[:, 448:512], pvacc[:, 512:576])
            state[qb] = (O_t, invs)
            for _ in weave:      # drain anything the kb loop didn't cover
                pass

        # Interleave: attention row qb needs keys 0..512*(qb+1) = subtiles
        # 0..qb, so row s can run right after phase_a(s). Projections run
        # two rows ahead (hides the full RoPE chain) and the deferred
        # norm/o_proj units are pushed into the late, gap-rich rows.
        import itertools
        for _ in phase_a_units(0):
            pass
        norm_sched = {s: [s - 1] for s in range(1, 8)}
        for s in range(NSB):
            w = []
            if s + 1 < NSB:
                w.append(phase_a_units(s + 1))
            for nb in norm_sched.get(s, []):
                w.append(norm_oproj_units(nb))
            attention(s, weave=itertools.chain(*w))
        # the last row normalizes itself via the per-qs tail pipeline


def build(L_=L, debug=False):
    nc = bacc.Bacc("TRN2", target_bir_lowering=False, debug=debug,
                   enable_asserts=False)
    aps = {}
    NSB = L_ // 512
    aps["xt"] = nc.dram_tensor("xt", [128, NSB * 4096], BF16,
                               kind="ExternalInput").ap()
    aps["wq"] = nc.dram_tensor("wq", [128, D], BF16,
                               kind="ExternalInput").ap()
    aps["wkv"] = nc.dram_tensor("wkv", [128, 2 * D], BF16,
                                kind="ExternalInput").ap()
    aps["wo"] = nc.dram_tensor("wo", [128, D], BF16,
                               kind="ExternalInput").ap()
    aps["cs"] = nc.dram_tensor("cs", [128, 2 * L_], BF16,
                               kind="ExternalInput").ap()
    aps["tip"] = nc.dram_tensor("tip", [128, 512], BF16,
                                kind="ExternalInput").ap()
    aps["partial"] = nc.dram_tensor("partial", [L_, D], BF16,
                                    kind="ExternalOutput").ap()

    with tile.TileContext(nc) as tc:
        emit(nc, tc, aps, L_)
    nc.compile()
    return nc, aps


def make_in_maps(x, Wq, Wk, Wv, Wo, L_=L):
    xr = _layout_x(x, L_)
    consts = _host_consts(L_)
    in_maps = []
    for c in range(N_CORES):
        wq, wkv, wo = _core_weights(c, Wq, Wk, Wv, Wo)
        m = {"xt": xr, "wq": wq, "wkv": wkv, "wo": wo}
        m.update(consts)
        in_maps.append(m)
    return in_maps


_CACHE = {}


def _run(inputs, trace=False, **kw):
    if trace:
        os.environ.pop("BASS_NEVER_TRACE", None)
    x = np.asarray(inputs["x"], np.float32)
    Wq = np.asarray(inputs["Wq"], np.float32)
    Wk = np.asarray(inputs["Wk"], np.float32)
    Wv = np.asarray(inputs["Wv"], np.float32)
    Wo = np.asarray(inputs["Wo"], np.float32)
    if "nc" not in _CACHE:
        _CACHE["nc"] = build()[0]
    nc = _CACHE["nc"]
    in_maps = make_in_maps(x, Wq, Wk, Wv, Wo)
    res = run_bass_kernel_spmd(nc, in_maps, core_ids=list(range(N_CORES)),
                               trace=trace, **kw)
    acc = np.zeros((L, D), np.float64)
    for r in res.results:
        acc += r["partial"].astype(np.float64)
    out = acc.astype(np.float32).reshape(B, L, D)
    return out, res


def kernel(**inputs):
    out, _ = _run(inputs)
    return out

